# revision 5
# baseline (speedup 1.0000x reference)
"""Multi-head attention (B=2, S=2048, H=1024, NH=16, HD=64) on 8 trn2 cores.

Sharding: tensor-parallel over heads. Core c owns heads {2c, 2c+1}, i.e.
feature columns [128c, 128c+128) of q/k/v. Wq/Wk/Wv are column-sharded,
Wo row-sharded; each core computes a full-shape partial output and the
host sums the 8 partials (the row-parallel reduce) during unshard.

On-chip layout is feature-major ("transposed"): the host passes
hsT = hidden_states.T so both matmul operands of every projection have
the contraction dim on partitions and no on-chip transposes of big
tensors are needed. Attention works on scoresT[tk, tq]; softmax's
normalizer comes from a ones-column augmented V matmul (exp is safe
without max-subtraction because scores are O(6) here).

All matmul operands are fp16 (PSUM accumulation stays fp32). Weights
arrive host-prearranged in the SBUF [partition, k-chunk, col] layout so
their DMA is contiguous. Attention matmuls are zero-padded to full
128x128 array shapes - half-array matmuls don't register for the PE's
HAM clock gate and the phase would run at 1.2GHz otherwise.

Schedule: attention is exp(ACT)-bound, so batch b+1's QKV projection
chains are interleaved into batch b's attention p-loop to fill the PE
slack, and the output projection of tq-block i is emitted during
tq-block i+1 so its operand wait never stalls the in-order PE queue.
Softmax normalization copies the Z row to partition 0 before
reciprocal_approx_fast (the custom DVE op misbehaves on inputs based
at partition 64) and drains both ctx accumulators before any further
work so the PSUM banks recycle immediately.

PSUM budget (8 banks): sc pool 2x[128,1024]f32 = 4, cx accumulators
2x[128,512]f32 = 2, shared pt pool (qkv pairs / transposes / outproj)
2x[128,512]f32 = 2.
"""

import numpy as np

B, S, H, NH, HD = 2, 2048, 1024, 16, 64
NCORES = 8
JC = 128  # head-columns per core (2 heads x 64)
T = B * S  # 4096 tokens
TQB = 512  # tq block
NKT = S // 128  # 16 tk blocks per batch
BASE = 10000.0

_nc_cache = [None]

_LDW_OPT = False


def _patch_ldw_opt():
    from concourse import bass_utils as _bu

    if getattr(_bu, "_ldw_patched", False):
        return
    _orig = _bu.run_command

    def _patched(argv, **kw):
        argv = [
            a.replace("--enable-ldw-opt=false", "--enable-ldw-opt=true")
            if _LDW_OPT and isinstance(a, str)
            else a
            for a in argv
        ]
        return _orig(argv, **kw)

    _bu.run_command = _patched
    _bu._ldw_patched = True


def _build():
    _patch_ldw_opt()
    import concourse.tile as tile
    from concourse import bacc, mybir
    from concourse.masks import make_identity

    F32 = mybir.dt.float32
    F16 = mybir.dt.float16
    EXP = mybir.ActivationFunctionType.Exp

    nc = bacc.Bacc("TRN2", target_bir_lowering=False, debug=False)

    hsT = nc.dram_tensor("hsT", [H, T], F16, kind="ExternalInput").ap()
    wqT = nc.dram_tensor("wqT", [128, 8, JC], F16, kind="ExternalInput").ap()
    wkT = nc.dram_tensor("wkT", [128, 8, JC], F16, kind="ExternalInput").ap()
    wvT = nc.dram_tensor("wvT", [128, 8, JC], F16, kind="ExternalInput").ap()
    woJI = nc.dram_tensor("woJI", [JC, H], F16, kind="ExternalInput").ap()
    cosT = nc.dram_tensor("cosT", [128, S], F16, kind="ExternalInput").ap()
    sinTs = nc.dram_tensor("sinTs", [128, S], F16, kind="ExternalInput").ap()
    out = nc.dram_tensor("out", [T, H], F32, kind="ExternalOutput").ap()

    with tile.TileContext(nc) as tc:
        with (
            tc.tile_pool(name="wts", bufs=1) as wts,
            tc.tile_pool(name="tabs", bufs=1) as tabs,
            tc.tile_pool(name="hst", bufs=20) as hst,
            tc.tile_pool(name="qkv", bufs=2) as qkvp,
            tc.tile_pool(name="sc", bufs=2, space="PSUM") as scp,
            tc.tile_pool(name="cxa", bufs=2, space="PSUM") as cxp,
            tc.tile_pool(name="pt", bufs=2, space="PSUM") as ptp,
            tc.tile_pool(name="rope", bufs=3) as ropep,
            tc.tile_pool(name="vaug", bufs=2) as vaugp,
            tc.tile_pool(name="expt", bufs=4) as exptp,
            tc.tile_pool(name="ctx", bufs=2) as ctxp,
            tc.tile_pool(name="nrm", bufs=2) as nrmp,
            tc.tile_pool(name="outs", bufs=4) as outsp,
            tc.tile_pool(name="zdr", bufs=4, space="DRAM") as zdrp,
        ):
            # ---- persistent weights / tables (contiguous DMAs) ----
            wq_sb = wts.tile([128, 8, JC], F16, tag="wq")
            nc.sync.dma_start(out=wq_sb[:], in_=wqT[:, :, :])
            wk_sb = wts.tile([128, 8, JC], F16, tag="wk")
            nc.sync.dma_start(out=wk_sb[:], in_=wkT[:, :, :])
            wv_sb = wts.tile([128, 8, JC], F16, tag="wv")
            nc.sync.dma_start(out=wv_sb[:], in_=wvT[:, :, :])
            wJ = wts.tile([128, H], F16, tag="wj")
            nc.sync.dma_start(out=wJ[:], in_=woJI[:, :])
            cos_sb = tabs.tile([128, S], F16, tag="cos")
            nc.sync.dma_start(out=cos_sb[:], in_=cosT[:, :])
            sin_sb = tabs.tile([128, S], F16, tag="sin")
            nc.sync.dma_start(out=sin_sb[:], in_=sinTs[:, :])
            ident = tabs.tile([128, 128], F32, tag="ident")
            make_identity(nc, ident[:])
            onesc = tabs.tile([128, NKT], F32, tag="ones")
            nc.vector.memset(onesc[:], 1.0)

            state = {}  # per-batch tiles

            def qkv_units(b):
                """List of closures; each emits one (2-chain) QKV unit."""
                qT = qkvp.tile([128, S], F16, tag="qT")
                kT = qkvp.tile([128, S], F16, tag="kT")
                vT = qkvp.tile([128, S], F32, tag="vT")
                state[b] = {"qT": qT, "kT": kT, "vT": vT}
                chains = []
                for nchi in range(S // TQB):
                    for kind, w_sb in (("q", wq_sb), ("k", wk_sb), ("v", wv_sb)):
                        chains.append((kind, w_sb, nchi))
                chunk_cache = {}

                def get_chunk(k, nchi):
                    if (k, nchi) not in chunk_cache:
                        t0 = b * S + nchi * TQB
                        c = hst.tile([128, TQB], F16, tag="hst")
                        nc.sync.dma_start(
                            out=c[:], in_=hsT[128 * k : 128 * (k + 1), t0 : t0 + TQB]
                        )
                        chunk_cache[(k, nchi)] = c
                    return chunk_cache[(k, nchi)]

                def make_unit(pair):
                    def emit():
                        pt_a = ptp.tile([128, TQB], F32, tag="pt")
                        pt_b = ptp.tile([128, TQB], F32, tag="pt")
                        ptiles = [pt_a, pt_b][: len(pair)]
                        for k in range(8):
                            for (kind, w_sb, nchi), p in zip(pair, ptiles):
                                nc.tensor.matmul(
                                    p[:], w_sb[:, k, :], get_chunk(k, nchi)[:],
                                    start=(k == 0), stop=(k == 7),
                                )
                        for (kind, w_sb, nchi), p in zip(pair, ptiles):
                            sl = slice(nchi * TQB, (nchi + 1) * TQB)
                            if kind == "v":
                                nc.vector.tensor_copy(vT[:, sl], p[:])
                                continue
                            dstT = qT if kind == "q" else kT
                            raw = ropep.tile([128, TQB], F16, tag="raw")
                            nc.vector.tensor_copy(raw[:], p[:])
                            rot = ropep.tile([128, TQB], F16, tag="rot")
                            for h0 in (0, 64):
                                nc.sync.dma_start(
                                    out=rot[h0 : h0 + 32, :],
                                    in_=raw[h0 + 32 : h0 + 64, :],
                                )
                                nc.sync.dma_start(
                                    out=rot[h0 + 32 : h0 + 64, :],
                                    in_=raw[h0 : h0 + 32, :],
                                )
                            t1 = ropep.tile([128, TQB], F16, tag="t1")
                            nc.vector.tensor_mul(t1[:], raw[:], cos_sb[:, sl])
                            t2 = ropep.tile([128, TQB], F16, tag="t2")
                            nc.vector.tensor_mul(t2[:], rot[:], sin_sb[:, sl])
                            nc.vector.tensor_add(dstT[:, sl], t1[:], t2[:])

                    return emit

                return [make_unit(chains[i0 : i0 + 2]) for i0 in range(0, len(chains), 2)]

            def setup(b):
                """kZA/kZB + augmented-V for batch b (after its QKV)."""
                st = state[b]
                kT, vT = st["kT"], st["vT"]
                kZA = qkvp.tile([128, S], F16, tag="kZA")
                nc.vector.memset(kZA[64:128, :], 0.0)
                nc.vector.tensor_copy(kZA[0:64, :], kT[0:64, :])
                kZB = qkvp.tile([128, S], F16, tag="kZB")
                nc.vector.memset(kZB[0:64, :], 0.0)
                nc.vector.tensor_copy(kZB[64:128, :], kT[64:128, :])
                vA = vaugp.tile([128, NKT, 128], F16, tag="vA")
                vB = vaugp.tile([128, NKT, 128], F16, tag="vB")
                nc.vector.memset(vA[:, :, 65:128], 0.0)
                nc.vector.memset(vB[:, :, 65:128], 0.0)
                nc.vector.tensor_copy(vA[:, :, 64], onesc[:])
                nc.vector.tensor_copy(vB[:, :, 64], onesc[:])
                for tkb in range(NKT):
                    pt = ptp.tile([128, TQB], F32, tag="pt")
                    nc.tensor.transpose(
                        pt[:, 0:128], vT[:, 128 * tkb : 128 * (tkb + 1)], ident[:]
                    )
                    nc.vector.tensor_copy(vA[:, tkb, 0:64], pt[:, 0:64])
                    nc.vector.tensor_copy(vB[:, tkb, 0:64], pt[:, 64:128])
                ctxS = ctxp.tile([128, S], F16, tag="cts")
                ctxB = ctxp.tile([64, S], F16, tag="ctb")
                st.update(
                    {"kZA": kZA, "kZB": kZB, "vA": vA, "vB": vB,
                     "ctxS": ctxS, "ctxB": ctxB}
                )

            def outproj(b, tqb):
                """Output projection for one tq block (4 row-blocks of 128)."""
                ctxS = state[b]["ctxS"]
                for tq8 in range(4 * tqb, 4 * (tqb + 1)):
                    csl = slice(128 * tq8, 128 * (tq8 + 1))
                    r0 = b * S + 128 * tq8
                    for ich in range(2):
                        isl = slice(ich * 512, (ich + 1) * 512)
                        po = ptp.tile([128, TQB], F32, tag="pt")
                        nc.tensor.matmul(
                            po[:], ctxS[:, csl], wJ[:, isl], start=True, stop=True
                        )
                        ot = outsp.tile([128, TQB], F32, tag="ot")
                        if (tq8 + ich) % 2 == 0:
                            nc.vector.tensor_copy(ot[:], po[:])
                        else:
                            nc.scalar.copy(ot[:], po[:])
                        nc.sync.dma_start(out=out[r0 : r0 + 128, isl], in_=ot[:])

            def norm(b, tqb, cxA, cxB):
                """Drain ctx accumulators, then softmax-normalize."""
                st = state[b]
                ctxS, ctxB = st["ctxS"], st["ctxB"]
                qsl = slice(tqb * TQB, (tqb + 1) * TQB)
                crawA = nrmp.tile([64, TQB], F32, tag="crawA")
                nc.vector.tensor_copy(crawA[:], cxA[0:64, :])
                zA = nrmp.tile([1, TQB], F32, tag="zA")
                nc.vector.tensor_copy(zA[:], cxA[64:65, :])
                crawB = nrmp.tile([64, TQB], F32, tag="crawB")
                nc.vector.tensor_copy(crawB[:], cxB[0:64, :])
                zB = nrmp.tile([1, TQB], F32, tag="zB")
                nc.vector.tensor_copy(zB[:], cxB[64:65, :])
                rzfB = nrmp.tile([1, TQB], F32, tag="rzfB")
                nc.vector.reciprocal_approx_fast(rzfB[:], zB[:])
                rzfA = nrmp.tile([1, TQB], F32, tag="rzfA")
                nc.vector.reciprocal_approx_fast(rzfA[:], zA[:])
                zdB = zdrp.tile([1, TQB], F32, tag="zdB")
                nc.sync.dma_start(out=zdB[:], in_=rzfB[:])
                zrepB = nrmp.tile([64, TQB], F32, tag="zrepB")
                nc.sync.dma_start(
                    out=zrepB[:], in_=zdB[0:1, :].to_broadcast([64, TQB])
                )
                zdA = zdrp.tile([1, TQB], F32, tag="zdA")
                nc.sync.dma_start(out=zdA[:], in_=rzfA[:])
                zrepA = nrmp.tile([64, TQB], F32, tag="zrepA")
                nc.sync.dma_start(
                    out=zrepA[:], in_=zdA[0:1, :].to_broadcast([64, TQB])
                )
                nc.vector.tensor_mul(ctxB[:, qsl], crawB[0:64, :], zrepB[:])
                nc.sync.dma_start(out=ctxS[64:128, qsl], in_=ctxB[:, qsl])
                nc.vector.tensor_mul(ctxS[0:64, qsl], crawA[0:64, :], zrepA[:])

            def attention(b, fill_units):
                """Attention for batch b; fill_units are interleaved QKV
                closures for the next batch (emitted into PE slack)."""
                st = state[b]
                qT, kZA, kZB = st["qT"], st["kZA"], st["kZB"]
                vA, vB = st["vA"], st["vB"]
                fills = list(fill_units)
                fi = 0
                for tqb in range(S // TQB):
                    qsl = slice(tqb * TQB, (tqb + 1) * TQB)
                    cxA = cxp.tile([128, TQB], F32, tag="cx")
                    cxB = cxp.tile([128, TQB], F32, tag="cx")
                    for p in range(NKT // 2):
                        scA = scp.tile([128, 2 * TQB], F32, tag="sc")
                        scB = scp.tile([128, 2 * TQB], F32, tag="sc")
                        for t in range(2):
                            tkb = 2 * p + t
                            ksl = slice(128 * tkb, 128 * (tkb + 1))
                            nc.tensor.matmul(
                                scA[:, t * TQB : (t + 1) * TQB],
                                kZA[:, ksl], qT[:, qsl],
                                start=True, stop=True,
                            )
                            nc.tensor.matmul(
                                scB[:, t * TQB : (t + 1) * TQB],
                                kZB[:, ksl], qT[:, qsl],
                                start=True, stop=True,
                            )
                        etA = exptp.tile([128, 2 * TQB], F16, tag="et")
                        nc.scalar.activation(etA[:], scA[:], EXP, scale=0.125)
                        etB = exptp.tile([128, 2 * TQB], F16, tag="et")
                        nc.scalar.activation(etB[:], scB[:], EXP, scale=0.125)
                        for t in range(2):
                            tkb = 2 * p + t
                            st_, sp_ = tkb == 0, tkb == NKT - 1
                            tsl = slice(t * TQB, (t + 1) * TQB)
                            nc.tensor.matmul(
                                cxA[:, :], vA[:, tkb, :], etA[:, tsl],
                                start=st_, stop=sp_,
                            )
                            nc.tensor.matmul(
                                cxB[:, :], vB[:, tkb, :], etB[:, tsl],
                                start=st_, stop=sp_,
                            )
                        if fi < len(fills):
                            fills[fi]()
                            fi += 1
                    if tqb > 0:
                        outproj(b, tqb - 1)
                    norm(b, tqb, cxA, cxB)
                while fi < len(fills):
                    fills[fi]()
                    fi += 1

            # ---------------- schedule ----------------
            units0 = qkv_units(0)
            for u in units0:
                u()
            setup(0)
            units1 = qkv_units(1)
            attention(0, units1)
            setup(1)
            outproj(0, 3)
            attention(1, [])
            outproj(1, 3)

    nc.compile()
    return nc


def _rope_tables():
    inv_freq = 1.0 / (BASE ** (np.arange(0, HD, 2, dtype=np.float64) / HD))
    t = np.arange(S, dtype=np.float64)
    freqs = np.outer(t, inv_freq)  # [S, 32]
    emb = np.concatenate([freqs, freqs], -1)  # [S, 64]
    cos = np.cos(emb).T.astype(np.float32)  # [64, S]
    sin = np.sin(emb).T.astype(np.float32)
    sin_signed = sin.copy()
    sin_signed[0:32] = -sin_signed[0:32]
    cosT = np.ascontiguousarray(np.tile(cos, (2, 1))).astype(np.float16)  # [128, S]
    sinTs = np.ascontiguousarray(np.tile(sin_signed, (2, 1))).astype(np.float16)
    return cosT, sinTs


def _prep_w(Wx, c):
    # host-side: [H, JC] column shard, rearranged to the SBUF layout
    # [partition, k-chunk, j] so the weight DMA is a single contiguous read
    sl = slice(JC * c, JC * (c + 1))
    wT = Wx[sl, :].T  # [H, JC]
    return np.ascontiguousarray(
        wT.reshape(8, 128, JC).transpose(1, 0, 2).astype(np.float16)
    )


def make_in_maps(hidden_states, Wq, Wk, Wv, Wo):
    hsT = np.ascontiguousarray(
        hidden_states.reshape(T, H).T.astype(np.float16)
    )  # [H, T]
    cosT, sinTs = _rope_tables()
    in_maps = []
    for c in range(NCORES):
        sl = slice(JC * c, JC * (c + 1))
        in_maps.append(
            {
                "hsT": hsT,
                "wqT": _prep_w(Wq, c),
                "wkT": _prep_w(Wk, c),
                "wvT": _prep_w(Wv, c),
                "woJI": np.ascontiguousarray(Wo[:, sl].T.astype(np.float16)),
                "cosT": cosT,
                "sinTs": sinTs,
            }
        )
    return in_maps


def kernel(hidden_states, Wq, Wk, Wv, Wo):
    hidden_states = np.asarray(hidden_states, np.float32)
    Wq, Wk, Wv, Wo = (np.asarray(w, np.float32) for w in (Wq, Wk, Wv, Wo))

    if _nc_cache[0] is None:
        _nc_cache[0] = _build()
    nc = _nc_cache[0]

    in_maps = make_in_maps(hidden_states, Wq, Wk, Wv, Wo)

    from concourse.bass_utils import run_bass_kernel_spmd

    res = run_bass_kernel_spmd(nc, in_maps, core_ids=list(range(NCORES)))
    acc = np.zeros((T, H), np.float64)
    for c in range(NCORES):
        acc += res.results[c]["out"]
    return acc.astype(np.float32).reshape(B, S, H)
